# revision 26
# baseline (speedup 1.0000x reference)
"""Causal multi-head self-attention (B=2, S=4096, D=1024, H=16, dk=64) on 8 trn2 cores.

Sharding: core c handles batch b = c // 4 and heads [4*(c%4) .. 4*(c%4)+3]
(data parallel on B, tensor parallel on heads / QKV / O projections).
Each core returns a partial [S, D] output; the host sums 4 partials per batch.

v2 pipeline (per core), mixed precision tuned to the 2e-2 rel-err gate:
  - Q/K projections: fp8e4 DoubleRow matmuls with two-level error feedback
    (x = xh + xl, W = wh + wl; terms xh*wh + xh*wl + xl*wh) -- ~bf16 accuracy
    at 1/4 the fp32r PE cost per pass.
  - V projection: single-level fp8 DoubleRow (32x-scaled Wv; softmax
    averaging washes out v quantization).
  - RoPE on DVE (bf16), outputs scattered head-contiguously by 16 small
    SBUF->SBUF DMAs per tile.
  - QK^T: bf16 (fp8 scores fail the error budget: softmax-averaged O shrinks
    while score error does not). Causal mask folded into the PE: an fp8
    DoubleRow identity x (+240 * -240) pattern accumulated into score PSUM.
  - exp: split between ACT (true exp -> fp8, scale=1/8 bias=-2.3125*ln2) and
    DVE "Schraudolph" (bits = clamp(1.4427*s' + 37.5) -> uint8 = fp8e4 bit
    pattern; the bias is a uniform es scale that cancels in normalization).
  - PV: fp8 DoubleRow, 2 key-blocks per instruction, 65th V column = ones
    gives the softmax denominator l.
  - normalize: l -> ACT copy (scale=32) -> DVE reciprocal -> gpsimd
    partition_broadcast -> DVE multiply into f32r ot.
  - Wo: f32r matmuls; y DMA'd to DRAM straight from PSUM (f32).
"""

import numpy as np
import ml_dtypes

import concourse.bass as bass
import concourse.bacc as bacc
import concourse.mybir as mybir
import concourse.tile as tile
from concourse.bass_utils import run_bass_kernel_spmd

P = 128
D_MODEL = 1024
N_HEADS = 16
D_K = 64
SEQ = 4096
BATCH = 2
N_CORES = 8
HEADS_PER_CORE = 4
ST = 512  # s-tile / q-tile width
THETA = 10000.0

E4 = ml_dtypes.float8_e4m3

f32 = mybir.dt.float32
f32r = mybir.dt.float32r
bf16 = mybir.dt.bfloat16
u8 = mybir.dt.uint8
fp8 = mybir.dt.float8e4
AF = mybir.ActivationFunctionType
OP = mybir.AluOpType
DR = mybir.MatmulPerfMode.DoubleRow

# exp path constants. Both paths must produce es = exp(s_true) * 2**-2.3125.
#   Wq, Wk are host-scaled by 16 (fp8 denormal floor: unscaled W std 0.031
#   sits in e4m3 denormal range where two-level quantization cannot recover
#   precision). scores in PSUM are s' = 16*16*8 * s_true = 2048 * s_true.
#   ACT:  exp(s' * ESCALE + ACT_BIAS)
#   DVE:  u8 bits = trunc(max(s' * SCH_A + SCH_B, 0)); es = 2**((bits-56)/8)
# bits = 8*log2(es) + 56 = 11.5416*s_true + 37.5 (+0.5 trunc-centering;
# the uniform es scale this implies cancels in the softmax denominator)
SSCALE = 2048.0
ESCALE = 1.0 / SSCALE
SCH_A = 8.0 * float(np.log2(np.e)) / SSCALE
SCH_B = 38.0
# bf16-es variant: u16 bits = 128*log2(es) + 16256, es scale C = 2**-2.3125
SCH16_A = 128.0 * float(np.log2(np.e)) / SSCALE
SCH16_B = 16256.0 - 128.0 * 2.3125 + 0.5
ES_BF16 = False  # True: es/V/PV in bf16 (no fp8 es grid noise, +PE cost)
ACT_BIAS = float(-2.3125 * np.log(2.0))
SCH_FRAC = 0.35       # fraction of exp groups routed to the DVE path
SCH_ONE_OP = False    # True: single tensor_scalar (relies on HW saturate-to-0)


def build_program(S=SEQ, reps=1, sch_frac=SCH_FRAC, sch_one_op=SCH_ONE_OP,
                  es_bf16=None):
    if es_bf16 is None:
        es_bf16 = ES_BF16
    nc = bacc.Bacc("TRN2", target_bir_lowering=False, debug=False,
                   num_devices=N_CORES)

    NT = S // ST
    NKB = S // P
    NIC = D_MODEL // P  # 8 contraction chunks of 128 over the model dim

    xh_d = nc.dram_tensor("xh", [D_MODEL, S], u8, kind="ExternalInput").ap()
    xl_d = nc.dram_tensor("xl", [D_MODEL, S], u8, kind="ExternalInput").ap()
    wh_d = nc.dram_tensor("wh", [D_MODEL, 768], u8, kind="ExternalInput").ap()
    wl_d = nc.dram_tensor("wl", [D_MODEL, 768], u8, kind="ExternalInput").ap()
    wo_d = nc.dram_tensor("woT", [256, D_MODEL], f32r, kind="ExternalInput").ap()
    cs_d = nc.dram_tensor("cs", [P, 2 * S], bf16, kind="ExternalInput").ap()
    mask_d = nc.dram_tensor("dmask", [P, 4 * 2 * ST], u8, kind="ExternalInput").ap()
    id_d = nc.dram_tensor("identm", [P, 2 * P], u8, kind="ExternalInput").ap()
    y_d = nc.dram_tensor("y", [S, D_MODEL], bf16, kind="ExternalOutput").ap()

    GK = 2

    with tile.TileContext(nc) as tc:
      for _rep in range(reps):
        with tc.tile_pool(name="res", bufs=1) as res:
            KT = res.tile([P, 2, S], bf16)      # rotated K^T, head-contiguous
            if es_bf16:
                Vb = res.tile([P, NKB, 384], bf16)  # V bf16, 96 cols/head
                vb4 = Vb.rearrange("p k (h c) -> p k h c", h=HEADS_PER_CORE)
                nc.vector.memset(Vb, 0.0)
                nc.vector.memset(vb4[:, :, :, 64:65], 1.0)
            else:
                # DoubleRow needs stationary free dim (out partitions) to be a
                # multiple of 32: pad the 65 (64 dims + ones) V cols to 96.
                Vh = res.tile([P, NKB, 384], u8)  # V fp8 hi plane, 96 cols/head
                Vl = res.tile([P, NKB, 384], u8)  # V fp8 lo plane (ones col 0)
                vh4 = Vh.rearrange("p k (h c) -> p k h c", h=HEADS_PER_CORE)
                vl4 = Vl.rearrange("p k (h c) -> p k h c", h=HEADS_PER_CORE)
                nc.vector.memset(Vh, 0)
                nc.vector.memset(Vl, 0)
                nc.vector.memset(vh4[:, :, :, 64:65], 0x38)  # fp8 1.0

            with tc.tile_pool(name="p1", bufs=2) as p1, \
                 tc.tile_pool(name="rot", bufs=2) as rotp, \
                 tc.tile_pool(name="p2", bufs=2) as p2, \
                 tc.tile_pool(name="ppp", bufs=2, space="PSUM") as ppp, \
                 tc.tile_pool(name="stg", bufs=2, space="PSUM") as stgp, \
                 tc.tile_pool(name="opp", bufs=2, space="PSUM") as opp:
                wh_sb = p1.tile([P, NIC, 768], u8, tag="wh", bufs=1)
                wh3 = wh_d.rearrange("(a p) f -> p a f", p=P)
                for _ic in range(NIC):
                    nc.sync.dma_start(wh_sb[:, _ic, :], wh3[:, _ic, :])
                wl_sb = p1.tile([P, NIC, 768], u8, tag="wl", bufs=1)
                wl3 = wl_d.rearrange("(a p) f -> p a f", p=P)
                for _ic in range(NIC):
                    nc.sync.dma_start(wl_sb[:, _ic, :], wl3[:, _ic, :])
                wo_sb = p1.tile([P, 2, D_MODEL], f32r, tag="wo", bufs=1)
                nc.sync.dma_start(wo_sb, wo_d.rearrange("(a p) f -> p a f", p=P))
                mask_sb = p2.tile([P, 4, 2, ST], u8, tag="mask", bufs=1)
                nc.sync.dma_start(
                    mask_sb, mask_d.rearrange("p (c j s) -> p c j s", c=4, j=2))
                id_sb = p2.tile([P, 2, P], u8, tag="ident", bufs=1)
                nc.sync.dma_start(id_sb, id_d.rearrange("p (j q) -> p j q", j=2))
                bias_t = p2.tile([P, 1], f32, tag="bias", bufs=1)
                nc.vector.memset(bias_t, ACT_BIAS)

                xh3 = xh_d.rearrange("(a p) s -> p a s", p=P)
                xl3 = xl_d.rearrange("(a p) s -> p a s", p=P)
                cs3 = cs_d.rearrange("p (c s) -> p c s", c=2)

                # deterministic ACT/DVE round-robin for exp groups
                sch_state = [0, 0]  # count, emitted-to-dve

                def use_dve_exp():
                    sch_state[0] += 1
                    want = int(np.floor(sch_state[0] * sch_frac))
                    if want > sch_state[1]:
                        sch_state[1] = want
                        return True
                    return False

                def emit_wo(qt_prev, ot_prev):
                    for sb in range(ST // P):
                        y_ps = stgp.tile([P, 2, ST], f32, tag="s")
                        for nh in (0, 1):
                            for j in (0, 1):
                                nc.tensor.matmul(
                                    y_ps[:, nh, :],
                                    lhsT=ot_prev[:, j, sb * P:(sb + 1) * P],
                                    rhs=wo_sb[:, j, nh * 512:(nh + 1) * 512],
                                    start=(j == 0), stop=(j == 1))
                        y_sb = p2.tile([P, 2, ST], bf16, tag="y", bufs=2)
                        nc.scalar.activation(y_sb, y_ps, AF.Copy)
                        nc.sync.dma_start(
                            y_d[qt_prev * ST + sb * P:
                                qt_prev * ST + (sb + 1) * P, :],
                            y_sb.rearrange("p a s -> p (a s)"))

                prev_ot = None
                for t in range(NT):
                    tsl = slice(t * ST, (t + 1) * ST)
                    xht = p1.tile([P, NIC, ST], u8, tag="xh", bufs=2)
                    nc.sync.dma_start(xht, xh3[:, :, tsl])
                    xlt = p1.tile([P, NIC, ST], u8, tag="xl", bufs=2)
                    nc.sync.dma_start(xlt, xl3[:, :, tsl])
                    cs = p1.tile([P, 2, ST], bf16, tag="cs")
                    nc.sync.dma_start(cs, cs3[:, :, tsl])

                    xh8 = xht.bitcast(fp8)
                    xl8 = xlt.bitcast(fp8)
                    wh8 = wh_sb.bitcast(fp8)
                    wl8 = wl_sb.bitcast(fp8)

                    # ---- Q/K projections (3-term fp8 DR feedback) + RoPE ----
                    qt_tile = p2.tile([P, 2, ST], bf16, tag="qt", bufs=2)
                    for dst, col0, dsl in ((qt_tile, 0, slice(0, ST)),
                                           (KT, 256, tsl)):
                        pe_ps = ppp.tile([P, ST], f32, tag="pp")
                        po_ps = ppp.tile([P, ST], f32, tag="pp")
                        for ps_t, base in ((pe_ps, col0), (po_ps, col0 + P)):
                            terms = []
                            for ic in range(0, NIC, 2):
                                icsl = slice(ic, ic + 2)
                                wcol = slice(base, base + P)
                                terms += [
                                    (wh_sb[:, icsl, wcol], xht[:, icsl, :]),
                                    (wh_sb[:, icsl, wcol], xlt[:, icsl, :]),
                                    (wl_sb[:, icsl, wcol], xht[:, icsl, :])]
                            for i, (lh, rh) in enumerate(terms):
                                nc.tensor.matmul(ps_t, lhsT=lh.bitcast(fp8),
                                                 rhs=rh.bitcast(fp8),
                                                 start=(i == 0),
                                                 stop=(i == len(terms) - 1),
                                                 perf_mode=DR)
                        # RoPE: rot_e = pe*cos - po*sin ; rot_o = pe*sin + po*cos
                        t1 = rotp.tile([P, ST], f32, tag="tmp", bufs=5)
                        t3 = rotp.tile([P, ST], f32, tag="tmp", bufs=5)
                        nc.vector.tensor_tensor(t1, pe_ps, cs[:, 0, :], OP.mult)
                        nc.vector.tensor_tensor(t3, pe_ps, cs[:, 1, :], OP.mult)
                        t2 = rotp.tile([P, ST], f32, tag="tmp", bufs=5)
                        t4 = rotp.tile([P, ST], f32, tag="tmp", bufs=5)
                        nc.vector.tensor_tensor(t2, po_ps, cs[:, 1, :], OP.mult)
                        nc.vector.tensor_tensor(t4, po_ps, cs[:, 0, :], OP.mult)
                        rot_e = rotp.tile([P, ST], bf16, tag="re")
                        rot_o = rotp.tile([P, ST], bf16, tag="ro")
                        nc.vector.tensor_tensor(rot_e, t1, t2, OP.subtract)
                        nc.vector.tensor_tensor(rot_o, t3, t4, OP.add)
                        # scatter: head h evens -> dst[(h%2)*64 + 0..32, h//2]
                        #          head h odds  -> dst[(h%2)*64 + 32..64, h//2]
                        for h in range(HEADS_PER_CORE):
                            j, hb = h // 2, (h % 2) * 64
                            nc.sync.dma_start(
                                dst[hb:hb + 32, j, dsl],
                                rot_e[h * 32:(h + 1) * 32, :])
                            nc.sync.dma_start(
                                dst[hb + 32:hb + 64, j, dsl],
                                rot_o[h * 32:(h + 1) * 32, :])

                    # ---- V projection (3-term fp8 DR feedback, 2-plane out) ----
                    for sb in range(ST // P):
                        kb = t * (ST // P) + sb
                        v_ps = ppp.tile([P, 256], f32, tag="pp")
                        terms = []
                        for ic in range(0, NIC, 2):
                            icsl = slice(ic, ic + 2)
                            ssl = slice(sb * P, (sb + 1) * P)
                            terms += [
                                (xht[:, icsl, ssl], wh_sb[:, icsl, 512:768]),
                                (xlt[:, icsl, ssl], wh_sb[:, icsl, 512:768]),
                                (xht[:, icsl, ssl], wl_sb[:, icsl, 512:768])]
                        for i, (lh, rh) in enumerate(terms):
                            nc.tensor.matmul(v_ps, lhsT=lh.bitcast(fp8),
                                             rhs=rh.bitcast(fp8),
                                             start=(i == 0),
                                             stop=(i == len(terms) - 1),
                                             perf_mode=DR)
                        v_ps4 = v_ps.rearrange("p (h c) -> p h c",
                                               h=HEADS_PER_CORE)
                        if es_bf16:
                            nc.scalar.activation(
                                vb4[:, kb, :, 0:64], v_ps4, AF.Copy)
                        else:
                            nc.scalar.activation(
                                vh4[:, kb, :, 0:64].bitcast(fp8), v_ps4, AF.Copy)
                            nc.vector.tensor_tensor(
                                vl4[:, kb, :, 0:64].bitcast(fp8), v_ps4,
                                vh4[:, kb, :, 0:64].bitcast(fp8), OP.subtract)

                    if prev_ot is not None:
                        emit_wo(t - 1, prev_ot)

                    # ---- attention for q-tile t ----
                    qt = t
                    nkb = (qt + 1) * (ST // P)
                    ot_acc = p2.tile([P, 2, ST], f32r, tag="ota", bufs=2)
                    for h in range(HEADS_PER_CORE):
                        j, hb = h // 2, (h % 2) * 64
                        o_ps = opp.tile([96, ST], f32, tag="o")
                        for g0 in range(0, nkb, GK):
                            q0 = 256 if g0 - qt * (ST // P) >= 2 else 0
                            stg = stgp.tile([P, GK, ST], f32, tag="s")
                            for gi in range(GK):
                                kb = g0 + gi
                                c = kb - qt * (ST // P)
                                nc.tensor.matmul(
                                    stg[:, gi, q0:],
                                    lhsT=KT[hb:hb + 64, j, kb * P:(kb + 1) * P],
                                    rhs=qt_tile[hb:hb + 64, j, q0:],
                                    start=True, stop=(c < 0))
                                if c >= 0:
                                    # in-PE causal mask: += I*(-240) @ (+240)pat
                                    nc.tensor.matmul(
                                        stg[:, gi, q0:],
                                        lhsT=id_sb.bitcast(fp8),
                                        rhs=mask_sb[:, c, :, q0:].bitcast(fp8),
                                        start=False, stop=True, perf_mode=DR)
                            if es_bf16:
                                es = p2.tile([P, GK, ST], bf16, tag="e", bufs=3)
                                if use_dve_exp():
                                    u16 = mybir.dt.uint16
                                    if sch_one_op:
                                        nc.vector.tensor_scalar(
                                            es[:, :, q0:].bitcast(u16),
                                            stg[:, :, q0:],
                                            SCH16_A, SCH16_B, OP.mult, OP.add)
                                    else:
                                        ty = rotp.tile([P, GK, ST], f32,
                                                       tag="ty", bufs=2)
                                        nc.vector.tensor_scalar(
                                            ty[:, :, q0:], stg[:, :, q0:],
                                            SCH16_A, SCH16_B, OP.mult, OP.add)
                                        nc.vector.tensor_scalar(
                                            es[:, :, q0:].bitcast(u16),
                                            ty[:, :, q0:], 0.0, None, OP.max)
                                else:
                                    nc.scalar.activation(
                                        es[:, :, q0:], stg[:, :, q0:],
                                        AF.Exp, bias=bias_t, scale=ESCALE)
                                for gi in range(GK):
                                    kb = g0 + gi
                                    nc.tensor.matmul(
                                        o_ps[:, q0:],
                                        lhsT=vb4[:, kb, h, :],
                                        rhs=es[:, gi, q0:],
                                        start=(kb == 0), stop=(kb == nkb - 1))
                            else:
                                es = p2.tile([P, GK, ST], u8, tag="e", bufs=3)
                                if use_dve_exp():
                                    if sch_one_op:
                                        nc.vector.tensor_scalar(
                                            es[:, :, q0:], stg[:, :, q0:],
                                            SCH_A, SCH_B, OP.mult, OP.add)
                                    else:
                                        ty = rotp.tile([P, GK, ST], f32,
                                                       tag="ty", bufs=2)
                                        nc.vector.tensor_scalar(
                                            ty[:, :, q0:], stg[:, :, q0:],
                                            SCH_A, SCH_B, OP.mult, OP.add)
                                        nc.vector.tensor_scalar(
                                            es[:, :, q0:], ty[:, :, q0:],
                                            0.0, None, OP.max)
                                else:
                                    nc.scalar.activation(
                                        es[:, :, q0:].bitcast(fp8),
                                        stg[:, :, q0:],
                                        AF.Exp, bias=bias_t, scale=ESCALE)
                                nc.tensor.matmul(
                                    o_ps[:, q0:],
                                    lhsT=vh4[:, g0:g0 + GK, h, :].bitcast(fp8),
                                    rhs=es[:, :, q0:].bitcast(fp8),
                                    start=(g0 == 0), stop=False,
                                    perf_mode=DR)
                                nc.tensor.matmul(
                                    o_ps[:, q0:],
                                    lhsT=vl4[:, g0:g0 + GK, h, :].bitcast(fp8),
                                    rhs=es[:, :, q0:].bitcast(fp8),
                                    start=False, stop=(g0 == nkb - GK),
                                    perf_mode=DR)
                        # ---- normalize: ot = o_ps[0:64] / (l) (32x folded) ----
                        lr_t = p2.tile([1, ST], f32, tag="lr", bufs=2)
                        nc.scalar.activation(lr_t, o_ps[64:65, :], AF.Copy,
                                             scale=32.0)
                        r_t = p2.tile([1, ST], f32, tag="rt", bufs=2)
                        nc.vector.reciprocal_approx_fast(r_t, lr_t)
                        rl = p2.tile([64, ST], f32, tag="rl", bufs=2)
                        nc.gpsimd.partition_broadcast(rl, r_t)
                        nc.vector.tensor_tensor(ot_acc[hb:hb + 64, j, :],
                                                o_ps[0:64, :], rl, OP.mult)

                    prev_ot = ot_acc

                emit_wo(NT - 1, prev_ot)

    nc.compile()
    return nc


def _round_fp32r(a):
    b = np.ascontiguousarray(a, dtype=np.float32).view(np.uint32)
    lsb = (b >> np.uint32(12)) & np.uint32(1)
    r = (b + np.uint32(0x7FF) + lsb) & np.uint32(0xFFFFF000)
    return r.view(np.float32)


def _fp8(a):
    return np.asarray(a, dtype=np.float32).astype(E4)


def make_core_inputs(x, token_positions, Wq, Wk, Wv, Wo, S=SEQ):
    """Host-side sharding/layout prep. Returns in_maps for the 8 cores."""
    x = np.asarray(x, dtype=np.float32)
    Wq = np.asarray(Wq, dtype=np.float32)
    Wk = np.asarray(Wk, dtype=np.float32)
    Wv = np.asarray(Wv, dtype=np.float32)
    Wo = np.asarray(Wo, dtype=np.float32)
    pos = np.asarray(token_positions).astype(np.float32)

    half = D_K // 2
    inv_freq = (1.0 / (np.float32(THETA) **
                       (np.arange(0, D_K, 2, dtype=np.float32) / np.float32(D_K))
                       )).astype(np.float32)
    freqs = pos[:, None] * inv_freq[None, :]          # [S, 32]
    cosT = np.cos(freqs).T.astype(np.float32)         # [32, S]
    sinT = np.sin(freqs).T.astype(np.float32)
    cos4 = np.tile(cosT, (HEADS_PER_CORE, 1))         # [128, S]
    sin4 = np.tile(sinT, (HEADS_PER_CORE, 1))
    cs = np.ascontiguousarray(
        np.concatenate([cos4, sin4], axis=1).astype(ml_dtypes.bfloat16))

    # diagonal-block mask patterns: +240 where masked (jq < i + 128c), j1 = 0
    ii = np.arange(P)[:, None]
    jj = np.arange(ST)[None, :]
    dmask = np.zeros((P, 4, 2, ST), np.float32)
    for c in range(4):
        dmask[:, c, 0, :] = (jj < ii + P * c) * 240.0
        dmask[:, c, 1, :] = dmask[:, c, 0, :]
    dmask = _fp8(dmask.reshape(P, 4 * 2 * ST)).view(np.uint8)
    dmask = np.ascontiguousarray(dmask)

    identm = np.zeros((P, 2, P), np.float32)
    identm[:, 0, :][np.arange(P), np.arange(P)] = -240.0
    identm[:, 1, :][np.arange(P), np.arange(P)] = -240.0
    identm = np.ascontiguousarray(_fp8(identm.reshape(P, 2 * P)).view(np.uint8))

    xTs = []
    for b in range(BATCH):
        xT = np.ascontiguousarray(x[b].T)             # [D, S]
        xh = _fp8(xT)
        xl = _fp8(xT - xh.astype(np.float32))
        xTs.append((xh.view(np.uint8), xl.view(np.uint8)))

    in_maps = []
    for c in range(N_CORES):
        b, g = c // 4, c % 4
        # Wq/Wk rows permuted: psum row 32*t+i = head (4g+t), even dim 2i
        # (evens block), odds at +1 (odds block).
        perm = np.empty(256, dtype=np.int64)
        for t in range(HEADS_PER_CORE):
            hg = HEADS_PER_CORE * g + t
            perm[t * half:(t + 1) * half] = hg * D_K + 2 * np.arange(half)
            perm[128 + t * half:128 + (t + 1) * half] = \
                hg * D_K + 2 * np.arange(half) + 1
        wqT = (16.0 * Wq[perm, :]).T                   # [D, 256] f32
        wkT = (16.0 * Wk[perm, :]).T
        wvT = (32.0 * Wv[g * 256:(g + 1) * 256, :]).T  # [D, 256]
        wall = np.concatenate([wqT, wkT, wvT], axis=1)  # [D, 768]
        wall_h = _fp8(wall)
        wall_l = _fp8(wall - wall_h.astype(np.float32))
        woT = np.ascontiguousarray(
            Wo[:, g * 256:(g + 1) * 256].T.astype(np.float32))
        xh, xl = xTs[b]
        in_maps.append({
            "xh": xh,
            "xl": xl,
            "wh": np.ascontiguousarray(wall_h.view(np.uint8)),
            "wl": np.ascontiguousarray(wall_l.view(np.uint8)),
            "woT": _round_fp32r(woT),
            "cs": cs,
            "dmask": dmask,
            "identm": identm,
        })
    return in_maps


_PROGRAM_CACHE = {}


def _get_program(S=SEQ):
    if S not in _PROGRAM_CACHE:
        _PROGRAM_CACHE[S] = build_program(S)
    return _PROGRAM_CACHE[S]


def run_cores(in_maps, trace=False, **kwargs):
    nc = _get_program(SEQ)
    return run_bass_kernel_spmd(nc, in_maps, core_ids=list(range(N_CORES)),
                                trace=trace, **kwargs)


def kernel(x, token_positions, Wq, Wk, Wv, Wo):
    in_maps = make_core_inputs(x, token_positions, Wq, Wk, Wv, Wo)
    res = run_cores(in_maps)
    out = np.zeros((BATCH, SEQ, D_MODEL), dtype=np.float32)
    for c in range(N_CORES):
        out[c // 4] += res.results[c]["y"].astype(np.float32)
    return out


_TIMED_CACHE = {}


def run_cores_timed(in_maps, iters=8, program=None):
    """Execute the SPMD program with device-resident inputs repeatedly and
    return (per-exec wall seconds list, outputs-per-core)."""
    import time

    import jax
    from jax.experimental.shard_map import shard_map
    from jax.sharding import Mesh, NamedSharding, PartitionSpec

    from concourse.bass2jax import (
        _bass_exec_p,
        install_neuronx_cc_hook,
        partition_id_tensor,
    )

    nc = program if program is not None else _get_program(SEQ)

    if id(nc) in _TIMED_CACHE:
        sharded, dev_in, out_avals, out_names, n_cores = _TIMED_CACHE[id(nc)]
        out = sharded(*dev_in)
        jax.block_until_ready(out)
        times = []
        for _ in range(iters):
            t0 = time.perf_counter()
            out = sharded(*dev_in)
            jax.block_until_ready(out)
            times.append(time.perf_counter() - t0)
        results = [
            {name: np.asarray(out[i]).reshape(n_cores, *out_avals[i].shape)[c]
             for i, name in enumerate(out_names)}
            for c in range(n_cores)
        ]
        return times, results
    install_neuronx_cc_hook()
    partition_name = nc.partition_id_tensor.name if nc.partition_id_tensor else None
    in_names, out_names, out_avals, zero_outs = [], [], [], []
    for alloc in nc.m.functions[0].allocations:
        if not isinstance(alloc, mybir.MemoryLocationSet):
            continue
        name = alloc.memorylocations[0].name
        if alloc.kind == "ExternalInput":
            if name != partition_name:
                in_names.append(name)
        elif alloc.kind == "ExternalOutput":
            out_names.append(name)
            shape = tuple(alloc.tensor_shape)
            dtype = mybir.dt.np(alloc.dtype)
            out_avals.append(jax.core.ShapedArray(shape, dtype))
            zero_outs.append(np.zeros(shape, dtype))
    n_params = len(in_names)
    all_names = in_names + out_names + ([partition_name] if partition_name else [])

    def _body(*args):
        operands = list(args)
        if partition_name:
            operands.append(partition_id_tensor())
        outs = _bass_exec_p.bind(
            *operands,
            out_avals=tuple(out_avals),
            in_names=tuple(all_names),
            out_names=tuple(out_names),
            lowering_input_output_aliases=(),
            sim_require_finite=True,
            sim_require_nnan=True,
            nc=nc,
        )
        return tuple(outs)

    n_cores = len(in_maps)
    devices = jax.devices()[:n_cores]
    mesh = Mesh(np.asarray(devices), ("core",))
    nin = n_params + len(out_names)
    sharded = jax.jit(
        shard_map(_body, mesh=mesh,
                  in_specs=(PartitionSpec("core"),) * nin,
                  out_specs=(PartitionSpec("core"),) * len(out_names),
                  check_rep=False),
        keep_unused=True)
    per_core = [[np.asarray(m[n]) for n in in_names] for m in in_maps]
    concat_in = [np.concatenate([per_core[c][i] for c in range(n_cores)], axis=0)
                 for i in range(n_params)]
    concat_zeros = [np.zeros((n_cores * z.shape[0], *z.shape[1:]), z.dtype)
                    for z in zero_outs]
    sh = NamedSharding(mesh, PartitionSpec("core"))
    dev_in = [jax.device_put(a, sh) for a in concat_in + concat_zeros]
    _TIMED_CACHE[id(nc)] = (sharded, dev_in, out_avals, out_names, n_cores)
    out = sharded(*dev_in)
    jax.block_until_ready(out)
    times = []
    for _ in range(iters):
        t0 = time.perf_counter()
        out = sharded(*dev_in)
        jax.block_until_ready(out)
        times.append(time.perf_counter() - t0)
    results = [
        {name: np.asarray(out[i]).reshape(n_cores, *out_avals[i].shape)[c]
         for i, name in enumerate(out_names)}
        for c in range(n_cores)
    ]
    return times, results
